# revision 52
# baseline (speedup 1.0000x reference)
"""Barrier-Net (DeepSets + barrier certificate) Trainium2 kernel.

Device kernel (per core, 8192 rows): feature-major ("transposed")
activations [features, batch] so every MLP layer is a single PE matmul
with weights as the stationary operand.  Per 512-row subchunk:
  - packed fp16 rows are DMA'd row-major, upconverted to fp32 on ACT,
    PE-transposed (2 matmul-transposes per 128-row block) into
    xt [128 feats, 512 rows] (feats = neigh 0:64 | obst 64:128).
  - phi layer 1 for all 16 neighbors / 32 obstacles: 24 matmuls with
    block-diagonal stacked weights -> PSUM [128, 512] (2 edges x 64 hidden).
  - relu(+bias) PSUM->SBUF split across ACT and DVE engines.
  - DeepSet sum + phi-L2 + rho-L1 collapsed into accumulating "fold"
    matmuls (W_eff = pnW2 @ rnW1); rho-L2 + psi-L1 likewise collapsed.
  - barrier terms via selection matmuls: pair-sum of squares -> sqrt ->
    (nrm-D)*nrm -> fast reciprocal -> broadcast-expand matmul -> weighted
    edge-sum matmul accumulated with the noise term.

Host/transfer strategy (the wall-clock bottleneck is the axon tunnel at
~45 MB/s with ~0.09s per-op round-trip latency):
  - the 128 neighbor/obstacle feature cols are shipped as int8 [B, 128]
    with a quantization scale folded into the host-built const blob
    (xt simply holds the integer codes; W1, D offsets and the -gamma
    edge-sum pick up powers of s) -- 8.4 MB on the wire instead of
    35.9 MB fp32 x + noise.  g/noise ship as fp16 [B, 4].  The fixed
    scale 2/127 covers the model's position range; if an input batch
    exceeds it, the scale adapts and the consts re-ship (slow but
    correct fallback).
  - the weight/const blob is cached on device and only re-shipped when
    the weights (or quant scale) actually change.
  - the donated output buffers are created on device by a tiny jitted
    zeros function (nothing shipped).
  - the shard_map executable is built once and cached (the stock
    run_bass_kernel_spmd path rebuilds + retraces it every call).
  - host-side packing/quantization runs in a small thread pool.
Sharding: pure data parallel, 8192 rows per NeuronCore, 8 cores.
"""

import os
import sys

import numpy as np

sys.path.insert(0, "/opt/trn_rl_repo")

import concourse.bass as bass  # noqa: E402
from concourse.bacc import Bacc  # noqa: E402
from concourse import mybir  # noqa: E402
from concourse.tile import TileContext  # noqa: E402

F32 = mybir.dt.float32
F16 = mybir.dt.float16
I8 = mybir.dt.int8
AF = mybir.ActivationFunctionType
OP = mybir.AluOpType

N_CORES = 8
B = 65536
N_CHUNKS = 1  # host-level chunks (2 was measured slower: dispatch overhead
              # of the extra call exceeds the pack/fetch overlap it buys)
BC = B // N_CHUNKS  # rows per chunk
RPC = BC // N_CORES  # rows per core per chunk
SUB = 512  # rows per subchunk
NSUB = RPC // SUB
NN, NO = 16, 32
D_ROBOT, D_OBST = 0.3, 0.5
B_GAMMA = 0.01
XC = 128  # quantized input cols: 64 neigh + 64 obst
S_DEFAULT = 2.0 / 127.0  # int8 quant scale covering the model's [-2, 2] range
# Coarser dedicated scale for the 32 neighbor-velocity cols (~0.1*N(0,1)):
# codes span ~±7, so the transport's zstd compression roughly halves those
# bytes on the wire.  Decode stays linear (scale folds into wn1 rows), so
# codes beyond ±7 are still exact -- only ±127 wrap matters for the range
# check, same as the position cols.
S_VEL = 0.55 / 7.0
_VEL_COLS = np.zeros(128, dtype=bool)
_VEL_COLS[np.arange(16) * 4 + 2] = True
_VEL_COLS[np.arange(16) * 4 + 3] = True

WEIGHT_NAMES = [
    "pnW1", "pnb1", "pnW2", "pnb2", "rnW1", "rnb1", "rnW2", "rnb2",
    "poW1", "pob1", "poW2", "pob2", "roW1", "rob1", "roW2", "rob2",
    "psW1", "psb1", "psW2", "psb2", "psW3", "psb3",
]

# const blob layout: (name, base_partition, n_partitions, n_cols)
_CONST_LAYOUT = [
    ("ident", 0, 128, 128),
    ("wn1", 0, 64, 8 * 128),
    ("wo1", 64, 64, 16 * 128),
    ("wne2", 0, 128, 64),
    ("woe2", 0, 128, 64),
    ("anao", 0, 128, 64),
    ("ag", 0, 2, 64),
    ("w2", 0, 64, 64),
    ("w3", 0, 64, 2),
    ("sel", 0, 128, 48),
    ("expand", 0, 48, 128),
    ("sumsel", 0, 128, 2),
    ("i2", 0, 2, 2),
    ("biasn", 0, 128, 1),
    ("biaso", 0, 128, 1),
    ("biasrho", 0, 128, 1),
    ("bpsi1", 0, 64, 1),
    ("bpsi2", 0, 64, 1),
    ("b3", 0, 2, 1),
    ("dap", 0, 48, 1),
]
_CONST_COLS = sum(c for (_, _, _, c) in _CONST_LAYOUT)
_CONST_OFF = {}
_off = 0
for _name, _bp, _np_, _c in _CONST_LAYOUT:
    _CONST_OFF[_name] = (_off, _bp, _np_, _c)
    _off += _c


def _build_const_blob(w, s=S_DEFAULT, sv=S_VEL):
    """Host-side packing of all weights/selectors into one [128, C] fp32 blob.

    ``s`` is the int8 quantization scale of the shipped position features:
    xt holds x/s, so phi-L1 weights absorb s, the barrier distance offsets
    divide by s, and the -gamma edge sum divides by s (prod = xt*rexp
    carries one s).  ``sv`` is the separate velocity-column scale, absorbed
    by the velocity rows of wn1 only (velocities feed nothing else).
    """
    blob = np.zeros((128, _CONST_COLS), dtype=np.float32)
    pn_scaled = np.array([s, s, sv, sv], np.float32)[:, None] * w["pnW1"]

    def put(name, arr):
        off, base, P, C = _CONST_OFF[name]
        a = np.asarray(arr, dtype=np.float32)
        assert a.shape == (P, C), (name, a.shape, (P, C))
        blob[base : base + P, off : off + C] = a

    put("ident", np.eye(128, dtype=np.float32))

    # phi_n L1: lhsT tile t computes hidden of neighbors (2t, 2t+1)
    wn1 = np.zeros((64, 8, 128), dtype=np.float32)
    for t in range(8):
        for j2 in range(2):
            j = 2 * t + j2
            wn1[4 * j : 4 * j + 4, t, 64 * j2 : 64 * j2 + 64] = pn_scaled
    put("wn1", wn1.reshape(64, 8 * 128))

    # phi_o L1: lhsT tile t computes hidden of obstacles (2t, 2t+1);
    # lives at partitions 64:128 to match the obstacle half of xT.
    wo1 = np.zeros((64, 16, 128), dtype=np.float32)
    for t in range(16):
        for j2 in range(2):
            k = 2 * t + j2
            wo1[2 * k : 2 * k + 2, t, 64 * j2 : 64 * j2 + 64] = s * w["poW1"]
    put("wo1", wo1.reshape(64, 16 * 128))

    # fold matmuls: phi-L2 and rho-L1 collapsed (both linear):
    # W_eff = pnW2 @ rnW1 [64,64]; stacked twice to sum the two 64-row halves.
    wne = w["pnW2"] @ w["rnW1"]
    woe = w["poW2"] @ w["roW1"]
    put("wne2", np.vstack([wne, wne]))
    put("woe2", np.vstack([woe, woe]))

    # rho-L2 + psi-L1 collapsed
    put("anao", np.vstack([w["rnW2"] @ w["psW1"][0:8], w["roW2"] @ w["psW1"][8:16]]))
    put("ag", w["psW1"][16:18])
    put("w2", w["psW2"])
    put("w3", w["psW3"])

    # barrier selectors (xT partition p = packed col p)
    sel = np.zeros((128, 48), dtype=np.float32)
    expand = np.zeros((48, 128), dtype=np.float32)
    sumsel = np.zeros((128, 2), dtype=np.float32)
    for j in range(NN):
        for c in range(2):
            sel[4 * j + c, j] = 1.0
            expand[j, 4 * j + c] = 1.0
            sumsel[4 * j + c, c] = -B_GAMMA / s
    for k in range(NO):
        for c in range(2):
            sel[64 + 2 * k + c, 16 + k] = 1.0
            expand[16 + k, 64 + 2 * k + c] = 1.0
            sumsel[64 + 2 * k + c, c] = -B_GAMMA / s
    put("sel", sel)
    put("expand", expand)
    put("sumsel", sumsel)
    put("i2", np.eye(2, dtype=np.float32))

    put("biasn", np.concatenate([w["pnb1"], w["pnb1"]])[:, None])
    put("biaso", np.concatenate([w["pob1"], w["pob1"]])[:, None])
    bn_eff = (NN * w["pnb2"]) @ w["rnW1"] + w["rnb1"]
    bo_eff = (NO * w["pob2"]) @ w["roW1"] + w["rob1"]
    put("biasrho", np.concatenate([bn_eff, bo_eff])[:, None])
    bpsi1 = w["rnb2"] @ w["psW1"][0:8] + w["rob2"] @ w["psW1"][8:16] + w["psb1"]
    put("bpsi1", bpsi1[:, None])
    put("bpsi2", w["psb2"][:, None])
    put("b3", w["psb3"][:, None])
    dap = np.concatenate(
        [np.full(NN, D_ROBOT / s, np.float32), np.full(NO, D_OBST / s, np.float32)]
    )
    put("dap", dap[:, None])
    return blob


def _build_bass():
    from contextlib import ExitStack

    nc = Bacc()
    xq_d = nc.dram_tensor("xq", [RPC, XC], I8, kind="ExternalInput")
    xg_d = nc.dram_tensor("xg", [RPC, 4], F16, kind="ExternalInput")
    cst_d = nc.dram_tensor("consts", [128, _CONST_COLS], F32, kind="ExternalInput")
    # fp16 consts: ag (psW1[16:18]) at cols 0:64, i2 identity at cols 64:66 —
    # lets the g / noise matmuls take the shipped fp16 data directly.
    csth_d = nc.dram_tensor("csth", [2, 66], F16, kind="ExternalInput")
    out_d = nc.dram_tensor("out", [RPC, 2], F16, kind="ExternalOutput")

    with TileContext(nc) as tc, ExitStack() as ctx:
        const = ctx.enter_context(tc.tile_pool(name="const", bufs=1))
        # bufs=NSUB on the DMA-touched pools: no slot reuse => the looped DMAs
        # carry at most one semaphore wait (hard ISA limit on DMA waits).
        xs_pool = ctx.enter_context(tc.tile_pool(name="xs", bufs=NSUB))
        xf_pool = ctx.enter_context(tc.tile_pool(name="xf", bufs=2))
        xt_pool = ctx.enter_context(tc.tile_pool(name="xt", bufs=2))
        r_pool = ctx.enter_context(tc.tile_pool(name="r", bufs=6))
        h_pool = ctx.enter_context(tc.tile_pool(name="h", bufs=2))
        b_pool = ctx.enter_context(tc.tile_pool(name="b", bufs=2))
        o_pool = ctx.enter_context(tc.tile_pool(name="o", bufs=2))
        od_pool = ctx.enter_context(tc.tile_pool(name="od", bufs=NSUB))
        ps_xt = ctx.enter_context(tc.tile_pool(name="ps_xt", bufs=2, space="PSUM"))
        ps_phi = ctx.enter_context(tc.tile_pool(name="ps_phi", bufs=2, space="PSUM"))
        ps_rho = ctx.enter_context(tc.tile_pool(name="ps_rho", bufs=1, space="PSUM"))
        ps_seq = ctx.enter_context(tc.tile_pool(name="ps_seq", bufs=2, space="PSUM"))
        ps_fin = ctx.enter_context(tc.tile_pool(name="ps_fin", bufs=1, space="PSUM"))

        cb = const.tile([128, _CONST_COLS], F32)
        nc.sync.dma_start(out=cb, in_=cst_d[:, :])

        def C(name):
            off, base, P, cols = _CONST_OFF[name]
            return cb[base : base + P, off : off + cols]

        ident = C("ident")

        # g / noise transposed, loaded once (strided DMA over the packed
        # tail cols), then upconverted fp16 -> fp32 on DVE.
        g16 = const.tile([2, RPC], F16)
        nz16 = const.tile([2, RPC], F16)
        nc.sync.dma_start(out=g16, in_=xg_d[:, 0:2].rearrange("n c -> c n"))
        nc.sync.dma_start(out=nz16, in_=xg_d[:, 2:4].rearrange("n c -> c n"))
        ch = const.tile([2, 66], F16)
        nc.sync.dma_start(out=ch, in_=csth_d[:, :])
        agh = ch[:, 0:64]
        i2h = ch[:, 64:66]

        # Prime ACT/DVE on the const blob so no later instruction needs to
        # carry both a DMA wait and a compute wait (PE transposes only have
        # one sync-wait slot; the PE primes are dummy transposes/matmuls).
        prime = const.tile([1, 2], F32)
        nc.scalar.copy(out=prime[:, 0:1], in_=cb[0:1, 0:1])
        nc.vector.tensor_copy(prime[:, 1:2], cb[0:1, 1:2])

        for s in range(NSUB):
            r0 = s * SUB
            # ---- load (int8) + upconvert + transpose x ----
            xs = xs_pool.tile([128, 4, XC], I8)
            nc.gpsimd.dma_start(
                out=xs, in_=xq_d[r0 : r0 + SUB, :].rearrange("(b p) f -> p b f", p=128)
            )
            xf = xf_pool.tile([128, 4, XC], F32, tag="xf")
            nc.scalar.copy(out=xf, in_=xs)
            xtn_ps = ps_xt.tile([64, SUB], F32, tag="xtps")
            xto_ps = ps_xt.tile([64, SUB], F32, tag="xtps")
            if s == 0:
                # dummy transpose / matmul: make PE observe the const-blob and
                # fp16-const DMAs with single-wait instructions before the
                # real consumers (which also carry compute waits) need them
                nc.tensor.transpose(
                    out=xtn_ps[0:1, 0:128], in_=cb[:, 0:1], identity=ident
                )
                nc.tensor.matmul(
                    xto_ps[0:2, 0:2], lhsT=i2h, rhs=i2h, start=True, stop=True
                )
            for b in range(4):
                nc.tensor.transpose(
                    out=xtn_ps[:, 128 * b : 128 * b + 128],
                    in_=xf[:, b, 0:64],
                    identity=ident,
                )
                nc.tensor.transpose(
                    out=xto_ps[:, 128 * b : 128 * b + 128],
                    in_=xf[:, b, 64:128],
                    identity=ident,
                )
            xt = xt_pool.tile([128, SUB], F32)
            nc.scalar.copy(out=xt[0:64, :], in_=xtn_ps)
            nc.scalar.copy(out=xt[64:128, :], in_=xto_ps)

            # ---- phi layer 1 + relu + fold ----
            rho_ps = ps_rho.tile([128, SUB], F32)
            relu_idx = 0
            for grp, ntile, wname, bname, fold_w, lo, hi in (
                ("n", 8, "wn1", "biasn", "wne2", 0, 64),
                ("o", 16, "wo1", "biaso", "woe2", 64, 128),
            ):
                wtile = C(wname)
                for t in range(ntile):
                    pp = ps_phi.tile([128, SUB], F32, tag="pp")
                    nc.tensor.matmul(
                        pp,
                        lhsT=wtile[:, 128 * t : 128 * t + 128],
                        rhs=xt[lo:hi, :],
                        start=True,
                        stop=True,
                    )
                    rt = r_pool.tile([128, SUB], F32, tag="rt")
                    if relu_idx % 2 == 0 or relu_idx == 23:
                        nc.scalar.activation(rt, pp, AF.Relu, bias=C(bname))
                    else:
                        nc.vector.tensor_scalar(
                            rt, pp, C(bname), 0.0, op0=OP.add, op1=OP.max
                        )
                    relu_idx += 1
                    nc.tensor.matmul(
                        rho_ps[lo:hi, :],
                        lhsT=C(fold_w),
                        rhs=rt,
                        start=(t == 0),
                        stop=(t == ntile - 1),
                        skip_group_check=True,
                    )

            H = h_pool.tile([128, SUB], F32, tag="H")
            nc.scalar.activation(H, rho_ps, AF.Relu, bias=C("biasrho"))

            # ---- barrier ----
            sq = b_pool.tile([128, SUB], F32, tag="sq")
            nc.vector.tensor_mul(sq, xt, xt)
            nrmsq_ps = ps_seq.tile([128, SUB], F32, tag="seq")
            nc.tensor.matmul(
                nrmsq_ps[0:48, :], lhsT=C("sel"), rhs=sq, start=True, stop=True
            )
            nrm = b_pool.tile([48, SUB], F32, tag="nrm")
            nc.scalar.activation(nrm, nrmsq_ps[0:48, :], AF.Sqrt)
            denom = b_pool.tile([48, SUB], F32, tag="denom")
            nc.vector.scalar_tensor_tensor(
                denom, nrm, C("dap"), nrm, op0=OP.subtract, op1=OP.mult
            )
            recip = b_pool.tile([48, SUB], F32, tag="recip")
            nc.vector.reciprocal_approx_fast(out=recip, in_=denom)
            rexp_ps = ps_seq.tile([128, SUB], F32, tag="seq")
            nc.tensor.matmul(
                rexp_ps, lhsT=C("expand"), rhs=recip, start=True, stop=True
            )
            prod = b_pool.tile([128, SUB], F32, tag="prod")
            nc.vector.tensor_mul(prod, xt, rexp_ps)

            fin_ps = ps_fin.tile([2, SUB], F32)
            nc.tensor.matmul(
                fin_ps, lhsT=C("sumsel"), rhs=prod, start=True, stop=False
            )
            nc.tensor.matmul(
                fin_ps,
                lhsT=i2h,
                rhs=nz16[:, r0 : r0 + SUB],
                start=False,
                stop=True,
                skip_group_check=True,
            )

            # ---- psi MLP ----
            psi1_ps = ps_seq.tile([128, SUB], F32, tag="seq")
            nc.tensor.matmul(
                psi1_ps[0:64, :], lhsT=C("anao"), rhs=H, start=True, stop=False
            )
            nc.tensor.matmul(
                psi1_ps[0:64, :],
                lhsT=agh,
                rhs=g16[:, r0 : r0 + SUB],
                start=False,
                stop=True,
                skip_group_check=True,
            )
            H1 = h_pool.tile([64, SUB], F32, tag="H1")
            nc.scalar.activation(H1, psi1_ps[0:64, :], AF.Relu, bias=C("bpsi1"))
            psi2_ps = ps_seq.tile([128, SUB], F32, tag="seq")
            nc.tensor.matmul(psi2_ps[0:64, :], lhsT=C("w2"), rhs=H1, start=True, stop=True)
            H2 = h_pool.tile([64, SUB], F32, tag="H2")
            nc.scalar.activation(H2, psi2_ps[0:64, :], AF.Relu, bias=C("bpsi2"))
            psi3_ps = ps_seq.tile([128, SUB], F32, tag="seq")
            nc.tensor.matmul(psi3_ps[0:2, :], lhsT=C("w3"), rhs=H2, start=True, stop=True)

            # ---- combine + output ----
            E = o_pool.tile([2, SUB], F32, tag="E")
            nc.scalar.activation(E, psi3_ps[0:2, :], AF.Tanh, bias=C("b3"))
            pre = o_pool.tile([2, SUB], F32, tag="pre")
            nc.vector.scalar_tensor_tensor(
                pre, E, 2.0, fin_ps, op0=OP.mult, op1=OP.add
            )
            a = o_pool.tile([2, SUB], F32, tag="a")
            nc.scalar.activation(a, pre, AF.Tanh)
            o = od_pool.tile([2, SUB], F16, tag="o")
            nc.vector.tensor_scalar(o, a, 2.0, None, op0=OP.mult)
            nc.gpsimd.dma_start(
                out=out_d[r0 : r0 + SUB, :].rearrange("n c -> c n"), in_=o
            )

    nc.finalize()
    return nc


_STATE = {}


_NWORK = 8
_WROWS = BC // _NWORK  # rows per pack worker within a chunk


def _range_block(args):
    x, lo, hi = args
    blk = x[lo:hi, 5:133]
    return float(blk.max()), float(blk.min())


def _quant_block(args):
    x, xq, tmp, inv_s, lo, hi, qlo = args
    t = tmp[qlo : qlo + (hi - lo)]
    np.multiply(x[lo:hi, 5:133], inv_s, out=t)
    np.rint(t, out=t)
    amax = max(float(t.max()), -float(t.min()))
    # no clip: t holds whole numbers; when amax <= 127 the unsafe cast is
    # exact, and when it is not, the caller discards this pack entirely.
    np.copyto(xq[qlo : qlo + (hi - lo)], t, casting="unsafe")
    return amax


def _chunk_range(st, x, c):
    """Max |value| over the quantized cols of chunk c (threaded)."""
    lo = c * BC
    mm = list(
        st["pool"].map(
            _range_block,
            [(x, lo + i * _WROWS, lo + (i + 1) * _WROWS) for i in range(_NWORK)],
        )
    )
    return max(max(h for h, _ in mm), -min(l for _, l in mm))


def _pack_chunk(st, x, noise, c, inv):
    """Quantize chunk c of x cols 5:133 to int8 [BC, 128] with per-column
    inverse scales ``inv`` (scalar or [128] vector); g + noise to fp16
    [BC, 4].  Also reports whether every value fit the code range
    (detected during quantization -- no separate range pass)."""
    lo = c * BC
    # reused buffers: calls are serial (the previous call's upload has
    # completed before its np.asarray returned), so overwriting is safe.
    # With multiple chunks the previous chunk's upload may still be in
    # flight, so only the single-chunk configuration may reuse.
    if N_CHUNKS == 1:
        xq = st["xq_buf"]
        tmp = st["tmp_buf"]
        xg = st["xg_buf"]
    else:
        xq = np.empty((BC, XC), dtype=np.int8)
        tmp = np.empty((BC, 128), dtype=np.float32)
        xg = np.empty((BC, 4), dtype=np.float16)
    amaxes = list(
        st["pool"].map(
            _quant_block,
            [
                (x, xq, tmp, inv, lo + i * _WROWS, lo + (i + 1) * _WROWS,
                 i * _WROWS)
                for i in range(_NWORK)
            ],
        )
    )
    xg[:, 0:2] = x[lo : lo + BC, 1:3]
    xg[:, 2:4] = noise[lo : lo + BC]
    return xq, xg, max(amaxes) <= 127.0


def _ensure_state():
    if _STATE:
        return _STATE
    import jax
    import jax.numpy as jnp
    from jax.sharding import Mesh, PartitionSpec, NamedSharding
    from jax.experimental.shard_map import shard_map
    from concourse.bass2jax import (
        _bass_exec_p,
        partition_id_tensor,
        install_neuronx_cc_hook,
    )

    nc = _build_bass()
    install_neuronx_cc_hook()

    partition_name = nc.partition_id_tensor.name if nc.partition_id_tensor else None
    in_names, out_names, out_avals = [], [], []
    for alloc in nc.m.functions[0].allocations:
        if not isinstance(alloc, mybir.MemoryLocationSet):
            continue
        name = alloc.memorylocations[0].name
        if alloc.kind == "ExternalInput":
            if name != partition_name:
                in_names.append(name)
        elif alloc.kind == "ExternalOutput":
            shape = tuple(alloc.tensor_shape)
            dtype = mybir.dt.np(alloc.dtype)
            out_names.append(name)
            out_avals.append(jax.core.ShapedArray(shape, dtype))
    n_params = len(in_names)
    n_outs = len(out_avals)
    in_names_full = in_names + out_names + (
        [partition_name] if partition_name else []
    )
    donate = tuple(range(n_params, n_params + n_outs))

    def _body(*args):
        operands = list(args)
        if partition_name is not None:
            operands.append(partition_id_tensor())
        outs = _bass_exec_p.bind(
            *operands,
            out_avals=tuple(out_avals),
            in_names=tuple(in_names_full),
            out_names=tuple(out_names),
            lowering_input_output_aliases=(),
            sim_require_finite=True,
            sim_require_nnan=True,
            nc=nc,
        )
        return tuple(outs)

    devices = jax.devices()[:N_CORES]
    assert len(devices) == N_CORES, (
        f"need {N_CORES} devices, have {len(jax.devices())}"
    )
    mesh = Mesh(np.asarray(devices), ("core",))
    sh = NamedSharding(mesh, PartitionSpec("core"))
    in_specs = (PartitionSpec("core"),) * (n_params + n_outs)
    out_specs = (PartitionSpec("core"),) * n_outs
    sharded = jax.jit(
        shard_map(
            _body, mesh=mesh, in_specs=in_specs, out_specs=out_specs, check_rep=False
        ),
        donate_argnums=donate,
        keep_unused=True,
    )
    # donated output buffers, created on device (nothing on the wire);
    # the kernel writes every output element so zeros vs junk is moot, but
    # zeros keep parity with the stock run_bass_kernel_spmd semantics.
    # One dispatch makes the buffers for all N_CHUNKS chunk calls.
    zeros_fn = jax.jit(
        lambda: tuple(
            jnp.zeros((N_CORES * a.shape[0],) + a.shape[1:], a.dtype)
            for _ in range(N_CHUNKS)
            for a in out_avals
        ),
        out_shardings=(sh,) * (n_outs * N_CHUNKS),
    )

    from concurrent.futures import ThreadPoolExecutor

    _STATE.update(
        nc=nc,
        jax=jax,
        sharded=sharded,
        zeros_fn=zeros_fn,
        n_outs=n_outs,
        sharding=sh,
        in_names=in_names,
        out_names=out_names,
        weights=None,
        scale=None,
        consts_dev=None,
        pool=ThreadPoolExecutor(_NWORK),
        xq_buf=np.empty((BC, XC), dtype=np.int8),
        tmp_buf=np.empty((BC, 128), dtype=np.float32),
        xg_buf=np.empty((BC, 4), dtype=np.float16),
        inv_vec=np.where(_VEL_COLS, 1.0 / S_VEL, 1.0 / S_DEFAULT).astype(
            np.float32
        ),
    )
    return _STATE


def _ensure_consts(st, inputs, s, sv):
    """Device-cache the const blob; re-ship only when weights/scales change."""
    w = {k: np.asarray(inputs[k]) for k in WEIGHT_NAMES}
    cached = st["weights"]
    if (
        cached is not None
        and st["scale"] == (s, sv)
        and all(np.array_equal(cached[k], w[k]) for k in WEIGHT_NAMES)
    ):
        return
    blob = _build_const_blob(w, s, sv)
    glob = np.ascontiguousarray(
        np.broadcast_to(blob, (N_CORES,) + blob.shape).reshape(
            N_CORES * 128, _CONST_COLS
        )
    )
    ch = np.zeros((2, 66), dtype=np.float16)
    ch[:, 0:64] = w["psW1"][16:18]
    ch[:, 64:66] = np.eye(2, dtype=np.float16)
    chg = np.ascontiguousarray(
        np.broadcast_to(ch, (N_CORES,) + ch.shape).reshape(N_CORES * 2, 66)
    )
    consts_dev = st["jax"].device_put(glob, st["sharding"])
    csth_dev = st["jax"].device_put(chg, st["sharding"])
    consts_dev.block_until_ready()
    csth_dev.block_until_ready()
    st["weights"] = w
    st["scale"] = (s, sv)
    st["consts_dev"] = consts_dev
    st["csth_dev"] = csth_dev


def _dispatch_chunks(st, x, noise, inputs, s, sv, inv, strict):
    """Pack + dispatch every chunk with scales (s, sv) / inverse-scale
    ``inv``.  Returns the list of per-chunk output tuples, or None if
    strict and a chunk exceeds the code range."""
    _ensure_consts(st, inputs, s, sv)
    zeros = st["zeros_fn"]()
    no = st["n_outs"]
    outs = []
    for c in range(N_CHUNKS):
        xq, xg, in_range = _pack_chunk(st, x, noise, c, inv)
        if strict and not in_range:
            return None
        args = {
            "xq": xq,
            "xg": xg,
            "consts": st["consts_dev"],
            "csth": st["csth_dev"],
        }
        outs.append(
            st["sharded"](
                *[args[n] for n in st["in_names"]],
                *zeros[c * no : (c + 1) * no],
            )
        )
    return outs


def _run_once(st, x, noise, inputs):
    outs = _dispatch_chunks(
        st, x, noise, inputs, S_DEFAULT, S_VEL, st["inv_vec"], strict=True
    )
    if outs is None:
        # batch exceeds the covered feature range: fall back to a single
        # widened uniform scale (consts re-fold + re-ship), redo every chunk.
        amax = max(_chunk_range(st, x, c) for c in range(N_CHUNKS))
        s = max(amax / 127.0, S_DEFAULT)
        outs = _dispatch_chunks(
            st, x, noise, inputs, s, s, 1.0 / s, strict=False
        )
    parts = [np.asarray(o[0]).astype(np.float32) for o in outs]
    return parts[0] if len(parts) == 1 else np.concatenate(parts, axis=0)


def kernel(**inputs):
    st = _ensure_state()
    x = np.asarray(inputs["x"])
    noise = np.asarray(inputs["noise"])
    try:
        return _run_once(st, x, noise, inputs)
    except Exception:
        # one retry: the axon-tunneled device occasionally reports a
        # transient unrecoverable-exec error that clears on the next run
        return _run_once(st, x, noise, inputs)


# revision 54
# speedup vs baseline: 1.0137x; 1.0137x over previous
"""Barrier-Net (DeepSets + barrier certificate) Trainium2 kernel.

Device kernel (per core, 8192 rows): feature-major ("transposed")
activations [features, batch] so every MLP layer is a single PE matmul
with weights as the stationary operand.  Per 512-row subchunk:
  - packed fp16 rows are DMA'd row-major, upconverted to fp32 on ACT,
    PE-transposed (2 matmul-transposes per 128-row block) into
    xt [128 feats, 512 rows] (feats = neigh 0:64 | obst 64:128).
  - phi layer 1 for all 16 neighbors / 32 obstacles: 24 matmuls with
    block-diagonal stacked weights -> PSUM [128, 512] (2 edges x 64 hidden).
  - relu(+bias) PSUM->SBUF split across ACT and DVE engines.
  - DeepSet sum + phi-L2 + rho-L1 collapsed into accumulating "fold"
    matmuls (W_eff = pnW2 @ rnW1); rho-L2 + psi-L1 likewise collapsed.
  - barrier terms via selection matmuls: pair-sum of squares -> sqrt ->
    (nrm-D)*nrm -> fast reciprocal -> broadcast-expand matmul -> weighted
    edge-sum matmul accumulated with the noise term.

Host/transfer strategy (the wall-clock bottleneck is the axon tunnel at
~45 MB/s with ~0.09s per-op round-trip latency):
  - the 128 neighbor/obstacle feature cols are shipped as int8 [B, 128]
    with a quantization scale folded into the host-built const blob
    (xt simply holds the integer codes; W1, D offsets and the -gamma
    edge-sum pick up powers of s) -- 8.4 MB on the wire instead of
    35.9 MB fp32 x + noise.  g/noise ship as fp16 [B, 4].  The fixed
    scale 2/127 covers the model's position range; if an input batch
    exceeds it, the scale adapts and the consts re-ship (slow but
    correct fallback).
  - the weight/const blob is cached on device and only re-shipped when
    the weights (or quant scale) actually change.
  - the donated output buffers are created on device by a tiny jitted
    zeros function (nothing shipped).
  - the shard_map executable is built once and cached (the stock
    run_bass_kernel_spmd path rebuilds + retraces it every call).
  - host-side packing/quantization runs in a small thread pool.
Sharding: pure data parallel, 8192 rows per NeuronCore, 8 cores.
"""

import os
import sys

import numpy as np

sys.path.insert(0, "/opt/trn_rl_repo")

import concourse.bass as bass  # noqa: E402
from concourse.bacc import Bacc  # noqa: E402
from concourse import mybir  # noqa: E402
from concourse.tile import TileContext  # noqa: E402

F32 = mybir.dt.float32
F16 = mybir.dt.float16
I8 = mybir.dt.int8
AF = mybir.ActivationFunctionType
OP = mybir.AluOpType

N_CORES = 8
B = 65536
N_CHUNKS = 1  # host-level chunks (2 was measured slower: dispatch overhead
              # of the extra call exceeds the pack/fetch overlap it buys)
BC = B // N_CHUNKS  # rows per chunk
RPC = BC // N_CORES  # rows per core per chunk
SUB = 512  # rows per subchunk
NSUB = RPC // SUB
NN, NO = 16, 32
D_ROBOT, D_OBST = 0.3, 0.5
B_GAMMA = 0.01
XC = 128  # quantized input cols: 64 neigh + 64 obst
S_DEFAULT = 2.0 / 127.0  # int8 quant scale covering the model's [-2, 2] range
# Coarser dedicated scale for the 32 neighbor-velocity cols (~0.1*N(0,1)):
# codes span ~±7, so the transport's zstd compression roughly halves those
# bytes on the wire.  Decode stays linear (scale folds into wn1 rows), so
# codes beyond ±7 are still exact -- only ±127 wrap matters for the range
# check, same as the position cols.
S_VEL = 0.55 / 7.0
_VEL_COLS = np.zeros(128, dtype=bool)
_VEL_COLS[np.arange(16) * 4 + 2] = True
_VEL_COLS[np.arange(16) * 4 + 3] = True

WEIGHT_NAMES = [
    "pnW1", "pnb1", "pnW2", "pnb2", "rnW1", "rnb1", "rnW2", "rnb2",
    "poW1", "pob1", "poW2", "pob2", "roW1", "rob1", "roW2", "rob2",
    "psW1", "psb1", "psW2", "psb2", "psW3", "psb3",
]

# const blob layout: (name, base_partition, n_partitions, n_cols)
_CONST_LAYOUT = [
    ("ident", 0, 128, 128),
    ("wn1", 0, 64, 8 * 128),
    ("wo1", 64, 64, 16 * 128),
    ("wne2", 0, 128, 64),
    ("woe2", 0, 128, 64),
    ("anao", 0, 128, 64),
    ("ag", 0, 2, 64),
    ("w2", 0, 64, 64),
    ("w3", 0, 64, 2),
    ("sel", 0, 128, 48),
    ("expand", 0, 48, 128),
    ("sumsel", 0, 128, 2),
    ("i2", 0, 2, 2),
    ("biasn", 0, 128, 1),
    ("biaso", 0, 128, 1),
    ("biasrho", 0, 128, 1),
    ("bpsi1", 0, 64, 1),
    ("bpsi2", 0, 64, 1),
    ("b3", 0, 2, 1),
    ("dap", 0, 48, 1),
]
_CONST_COLS = sum(c for (_, _, _, c) in _CONST_LAYOUT)
_CONST_OFF = {}
_off = 0
for _name, _bp, _np_, _c in _CONST_LAYOUT:
    _CONST_OFF[_name] = (_off, _bp, _np_, _c)
    _off += _c


def _build_const_blob(w, s=S_DEFAULT, sv=S_VEL):
    """Host-side packing of all weights/selectors into one [128, C] fp32 blob.

    ``s`` is the int8 quantization scale of the shipped position features:
    xt holds x/s, so phi-L1 weights absorb s, the barrier distance offsets
    divide by s, and the -gamma edge sum divides by s (prod = xt*rexp
    carries one s).  ``sv`` is the separate velocity-column scale, absorbed
    by the velocity rows of wn1 only (velocities feed nothing else).
    """
    blob = np.zeros((128, _CONST_COLS), dtype=np.float32)
    pn_scaled = np.array([s, s, sv, sv], np.float32)[:, None] * w["pnW1"]

    def put(name, arr):
        off, base, P, C = _CONST_OFF[name]
        a = np.asarray(arr, dtype=np.float32)
        assert a.shape == (P, C), (name, a.shape, (P, C))
        blob[base : base + P, off : off + C] = a

    put("ident", np.eye(128, dtype=np.float32))

    # phi_n L1: lhsT tile t computes hidden of neighbors (2t, 2t+1)
    wn1 = np.zeros((64, 8, 128), dtype=np.float32)
    for t in range(8):
        for j2 in range(2):
            j = 2 * t + j2
            wn1[4 * j : 4 * j + 4, t, 64 * j2 : 64 * j2 + 64] = pn_scaled
    put("wn1", wn1.reshape(64, 8 * 128))

    # phi_o L1: lhsT tile t computes hidden of obstacles (2t, 2t+1);
    # lives at partitions 64:128 to match the obstacle half of xT.
    wo1 = np.zeros((64, 16, 128), dtype=np.float32)
    for t in range(16):
        for j2 in range(2):
            k = 2 * t + j2
            wo1[2 * k : 2 * k + 2, t, 64 * j2 : 64 * j2 + 64] = s * w["poW1"]
    put("wo1", wo1.reshape(64, 16 * 128))

    # fold matmuls: phi-L2 and rho-L1 collapsed (both linear):
    # W_eff = pnW2 @ rnW1 [64,64]; stacked twice to sum the two 64-row halves.
    wne = w["pnW2"] @ w["rnW1"]
    woe = w["poW2"] @ w["roW1"]
    put("wne2", np.vstack([wne, wne]))
    put("woe2", np.vstack([woe, woe]))

    # rho-L2 + psi-L1 collapsed
    put("anao", np.vstack([w["rnW2"] @ w["psW1"][0:8], w["roW2"] @ w["psW1"][8:16]]))
    put("ag", w["psW1"][16:18])
    put("w2", w["psW2"])
    put("w3", w["psW3"])

    # barrier selectors (xT partition p = packed col p)
    sel = np.zeros((128, 48), dtype=np.float32)
    expand = np.zeros((48, 128), dtype=np.float32)
    sumsel = np.zeros((128, 2), dtype=np.float32)
    for j in range(NN):
        for c in range(2):
            sel[4 * j + c, j] = 1.0
            expand[j, 4 * j + c] = 1.0
            sumsel[4 * j + c, c] = -B_GAMMA / s
    for k in range(NO):
        for c in range(2):
            sel[64 + 2 * k + c, 16 + k] = 1.0
            expand[16 + k, 64 + 2 * k + c] = 1.0
            sumsel[64 + 2 * k + c, c] = -B_GAMMA / s
    put("sel", sel)
    put("expand", expand)
    put("sumsel", sumsel)
    put("i2", np.eye(2, dtype=np.float32))

    put("biasn", np.concatenate([w["pnb1"], w["pnb1"]])[:, None])
    put("biaso", np.concatenate([w["pob1"], w["pob1"]])[:, None])
    bn_eff = (NN * w["pnb2"]) @ w["rnW1"] + w["rnb1"]
    bo_eff = (NO * w["pob2"]) @ w["roW1"] + w["rob1"]
    put("biasrho", np.concatenate([bn_eff, bo_eff])[:, None])
    bpsi1 = w["rnb2"] @ w["psW1"][0:8] + w["rob2"] @ w["psW1"][8:16] + w["psb1"]
    put("bpsi1", bpsi1[:, None])
    put("bpsi2", w["psb2"][:, None])
    put("b3", w["psb3"][:, None])
    dap = np.concatenate(
        [np.full(NN, D_ROBOT / s, np.float32), np.full(NO, D_OBST / s, np.float32)]
    )
    put("dap", dap[:, None])
    return blob


def _build_bass():
    from contextlib import ExitStack

    nc = Bacc()
    xq_d = nc.dram_tensor("xq", [RPC, XC], I8, kind="ExternalInput")
    xg_d = nc.dram_tensor("xg", [RPC, 4], F16, kind="ExternalInput")
    cst_d = nc.dram_tensor("consts", [128, _CONST_COLS], F32, kind="ExternalInput")
    # fp16 consts: ag (psW1[16:18]) at cols 0:64, i2 identity at cols 64:66 —
    # lets the g / noise matmuls take the shipped fp16 data directly.
    csth_d = nc.dram_tensor("csth", [2, 66], F16, kind="ExternalInput")
    out_d = nc.dram_tensor("out", [RPC, 2], F16, kind="ExternalOutput")

    with TileContext(nc) as tc, ExitStack() as ctx:
        const = ctx.enter_context(tc.tile_pool(name="const", bufs=1))
        # bufs=NSUB on the DMA-touched pools: no slot reuse => the looped DMAs
        # carry at most one semaphore wait (hard ISA limit on DMA waits).
        xs_pool = ctx.enter_context(tc.tile_pool(name="xs", bufs=NSUB))
        xf_pool = ctx.enter_context(tc.tile_pool(name="xf", bufs=2))
        xt_pool = ctx.enter_context(tc.tile_pool(name="xt", bufs=2))
        r_pool = ctx.enter_context(tc.tile_pool(name="r", bufs=6))
        h_pool = ctx.enter_context(tc.tile_pool(name="h", bufs=2))
        b_pool = ctx.enter_context(tc.tile_pool(name="b", bufs=2))
        o_pool = ctx.enter_context(tc.tile_pool(name="o", bufs=2))
        od_pool = ctx.enter_context(tc.tile_pool(name="od", bufs=NSUB))
        ps_xt = ctx.enter_context(tc.tile_pool(name="ps_xt", bufs=2, space="PSUM"))
        ps_phi = ctx.enter_context(tc.tile_pool(name="ps_phi", bufs=2, space="PSUM"))
        ps_rho = ctx.enter_context(tc.tile_pool(name="ps_rho", bufs=1, space="PSUM"))
        ps_seq = ctx.enter_context(tc.tile_pool(name="ps_seq", bufs=2, space="PSUM"))
        ps_fin = ctx.enter_context(tc.tile_pool(name="ps_fin", bufs=1, space="PSUM"))

        cb = const.tile([128, _CONST_COLS], F32)
        nc.sync.dma_start(out=cb, in_=cst_d[:, :])

        def C(name):
            off, base, P, cols = _CONST_OFF[name]
            return cb[base : base + P, off : off + cols]

        ident = C("ident")

        # g / noise transposed, loaded once (strided DMA over the packed
        # tail cols), then upconverted fp16 -> fp32 on DVE.
        g16 = const.tile([2, RPC], F16)
        nz16 = const.tile([2, RPC], F16)
        nc.sync.dma_start(out=g16, in_=xg_d[:, 0:2].rearrange("n c -> c n"))
        nc.sync.dma_start(out=nz16, in_=xg_d[:, 2:4].rearrange("n c -> c n"))
        ch = const.tile([2, 66], F16)
        nc.sync.dma_start(out=ch, in_=csth_d[:, :])
        agh = ch[:, 0:64]
        i2h = ch[:, 64:66]

        # Prime ACT/DVE on the const blob so no later instruction needs to
        # carry both a DMA wait and a compute wait (PE transposes only have
        # one sync-wait slot; the PE primes are dummy transposes/matmuls).
        prime = const.tile([1, 2], F32)
        nc.scalar.copy(out=prime[:, 0:1], in_=cb[0:1, 0:1])
        nc.vector.tensor_copy(prime[:, 1:2], cb[0:1, 1:2])

        for s in range(NSUB):
            r0 = s * SUB
            # ---- load (int8) + upconvert + transpose x ----
            xs = xs_pool.tile([128, 4, XC], I8)
            nc.gpsimd.dma_start(
                out=xs, in_=xq_d[r0 : r0 + SUB, :].rearrange("(b p) f -> p b f", p=128)
            )
            xf = xf_pool.tile([128, 4, XC], F32, tag="xf")
            nc.scalar.copy(out=xf, in_=xs)
            xtn_ps = ps_xt.tile([64, SUB], F32, tag="xtps")
            xto_ps = ps_xt.tile([64, SUB], F32, tag="xtps")
            if s == 0:
                # dummy transpose / matmul: make PE observe the const-blob and
                # fp16-const DMAs with single-wait instructions before the
                # real consumers (which also carry compute waits) need them
                nc.tensor.transpose(
                    out=xtn_ps[0:1, 0:128], in_=cb[:, 0:1], identity=ident
                )
                nc.tensor.matmul(
                    xto_ps[0:2, 0:2], lhsT=i2h, rhs=i2h, start=True, stop=True
                )
            for b in range(4):
                nc.tensor.transpose(
                    out=xtn_ps[:, 128 * b : 128 * b + 128],
                    in_=xf[:, b, 0:64],
                    identity=ident,
                )
                nc.tensor.transpose(
                    out=xto_ps[:, 128 * b : 128 * b + 128],
                    in_=xf[:, b, 64:128],
                    identity=ident,
                )
            xt = xt_pool.tile([128, SUB], F32)
            nc.scalar.copy(out=xt[0:64, :], in_=xtn_ps)
            nc.scalar.copy(out=xt[64:128, :], in_=xto_ps)

            # ---- phi layer 1 + relu + fold ----
            rho_ps = ps_rho.tile([128, SUB], F32)
            relu_idx = 0
            for grp, ntile, wname, bname, fold_w, lo, hi in (
                ("n", 8, "wn1", "biasn", "wne2", 0, 64),
                ("o", 16, "wo1", "biaso", "woe2", 64, 128),
            ):
                wtile = C(wname)
                for t in range(ntile):
                    pp = ps_phi.tile([128, SUB], F32, tag="pp")
                    nc.tensor.matmul(
                        pp,
                        lhsT=wtile[:, 128 * t : 128 * t + 128],
                        rhs=xt[lo:hi, :],
                        start=True,
                        stop=True,
                    )
                    rt = r_pool.tile([128, SUB], F32, tag="rt")
                    if relu_idx % 2 == 0 or relu_idx == 23:
                        nc.scalar.activation(rt, pp, AF.Relu, bias=C(bname))
                    else:
                        nc.vector.tensor_scalar(
                            rt, pp, C(bname), 0.0, op0=OP.add, op1=OP.max
                        )
                    relu_idx += 1
                    nc.tensor.matmul(
                        rho_ps[lo:hi, :],
                        lhsT=C(fold_w),
                        rhs=rt,
                        start=(t == 0),
                        stop=(t == ntile - 1),
                        skip_group_check=True,
                    )

            H = h_pool.tile([128, SUB], F32, tag="H")
            nc.scalar.activation(H, rho_ps, AF.Relu, bias=C("biasrho"))

            # ---- barrier ----
            sq = b_pool.tile([128, SUB], F32, tag="sq")
            nc.vector.tensor_mul(sq, xt, xt)
            nrmsq_ps = ps_seq.tile([128, SUB], F32, tag="seq")
            nc.tensor.matmul(
                nrmsq_ps[0:48, :], lhsT=C("sel"), rhs=sq, start=True, stop=True
            )
            nrm = b_pool.tile([48, SUB], F32, tag="nrm")
            nc.scalar.activation(nrm, nrmsq_ps[0:48, :], AF.Sqrt)
            denom = b_pool.tile([48, SUB], F32, tag="denom")
            nc.vector.scalar_tensor_tensor(
                denom, nrm, C("dap"), nrm, op0=OP.subtract, op1=OP.mult
            )
            recip = b_pool.tile([48, SUB], F32, tag="recip")
            nc.vector.reciprocal_approx_fast(out=recip, in_=denom)
            rexp_ps = ps_seq.tile([128, SUB], F32, tag="seq")
            nc.tensor.matmul(
                rexp_ps, lhsT=C("expand"), rhs=recip, start=True, stop=True
            )
            prod = b_pool.tile([128, SUB], F32, tag="prod")
            nc.vector.tensor_mul(prod, xt, rexp_ps)

            fin_ps = ps_fin.tile([2, SUB], F32)
            nc.tensor.matmul(
                fin_ps, lhsT=C("sumsel"), rhs=prod, start=True, stop=False
            )
            nc.tensor.matmul(
                fin_ps,
                lhsT=i2h,
                rhs=nz16[:, r0 : r0 + SUB],
                start=False,
                stop=True,
                skip_group_check=True,
            )

            # ---- psi MLP ----
            psi1_ps = ps_seq.tile([128, SUB], F32, tag="seq")
            nc.tensor.matmul(
                psi1_ps[0:64, :], lhsT=C("anao"), rhs=H, start=True, stop=False
            )
            nc.tensor.matmul(
                psi1_ps[0:64, :],
                lhsT=agh,
                rhs=g16[:, r0 : r0 + SUB],
                start=False,
                stop=True,
                skip_group_check=True,
            )
            H1 = h_pool.tile([64, SUB], F32, tag="H1")
            nc.scalar.activation(H1, psi1_ps[0:64, :], AF.Relu, bias=C("bpsi1"))
            psi2_ps = ps_seq.tile([128, SUB], F32, tag="seq")
            nc.tensor.matmul(psi2_ps[0:64, :], lhsT=C("w2"), rhs=H1, start=True, stop=True)
            H2 = h_pool.tile([64, SUB], F32, tag="H2")
            nc.scalar.activation(H2, psi2_ps[0:64, :], AF.Relu, bias=C("bpsi2"))
            psi3_ps = ps_seq.tile([128, SUB], F32, tag="seq")
            nc.tensor.matmul(psi3_ps[0:2, :], lhsT=C("w3"), rhs=H2, start=True, stop=True)

            # ---- combine + output ----
            E = o_pool.tile([2, SUB], F32, tag="E")
            nc.scalar.activation(E, psi3_ps[0:2, :], AF.Tanh, bias=C("b3"))
            pre = o_pool.tile([2, SUB], F32, tag="pre")
            nc.vector.scalar_tensor_tensor(
                pre, E, 2.0, fin_ps, op0=OP.mult, op1=OP.add
            )
            a = o_pool.tile([2, SUB], F32, tag="a")
            nc.scalar.activation(a, pre, AF.Tanh)
            o = od_pool.tile([2, SUB], F16, tag="o")
            nc.vector.tensor_scalar(o, a, 2.0, None, op0=OP.mult)
            nc.gpsimd.dma_start(
                out=out_d[r0 : r0 + SUB, :].rearrange("n c -> c n"), in_=o
            )

    nc.finalize()
    return nc


_STATE = {}


_NWORK = 8
_WROWS = BC // _NWORK  # rows per pack worker within a chunk


def _range_block(args):
    x, lo, hi = args
    blk = x[lo:hi, 5:133]
    return float(blk.max()), float(blk.min())


def _quant_block(args):
    x, xq, tmp, inv_s, lo, hi, qlo = args
    t = tmp[qlo : qlo + (hi - lo)]
    np.multiply(x[lo:hi, 5:133], inv_s, out=t)
    np.rint(t, out=t)
    amax = max(float(t.max()), -float(t.min()))
    # no clip: t holds whole numbers; when amax <= 127 the unsafe cast is
    # exact, and when it is not, the caller discards this pack entirely.
    np.copyto(xq[qlo : qlo + (hi - lo)], t, casting="unsafe")
    return amax


def _chunk_range(st, x, c):
    """Max |value| over the quantized cols of chunk c (threaded)."""
    lo = c * BC
    mm = list(
        st["pool"].map(
            _range_block,
            [(x, lo + i * _WROWS, lo + (i + 1) * _WROWS) for i in range(_NWORK)],
        )
    )
    return max(max(h for h, _ in mm), -min(l for _, l in mm))


def _pack_chunk(st, x, noise, c, inv):
    """Quantize chunk c of x cols 5:133 to int8 [BC, 128] with per-column
    inverse scales ``inv`` (scalar or [128] vector); g + noise to fp16
    [BC, 4].  Also reports whether every value fit the code range
    (detected during quantization -- no separate range pass)."""
    lo = c * BC
    # reused buffers: calls are serial (the previous call's upload has
    # completed before its np.asarray returned), so overwriting is safe.
    # With multiple chunks the previous chunk's upload may still be in
    # flight, so only the single-chunk configuration may reuse.
    if N_CHUNKS == 1:
        xq = st["xq_buf"]
        tmp = st["tmp_buf"]
        xg = st["xg_buf"]
    else:
        xq = np.empty((BC, XC), dtype=np.int8)
        tmp = np.empty((BC, 128), dtype=np.float32)
        xg = np.empty((BC, 4), dtype=np.float16)
    amaxes = list(
        st["pool"].map(
            _quant_block,
            [
                (x, xq, tmp, inv, lo + i * _WROWS, lo + (i + 1) * _WROWS,
                 i * _WROWS)
                for i in range(_NWORK)
            ],
        )
    )
    xg[:, 0:2] = x[lo : lo + BC, 1:3]
    xg[:, 2:4] = noise[lo : lo + BC]
    return xq, xg, max(amaxes) <= 127.0


def _ensure_state():
    if _STATE:
        return _STATE
    import jax
    import jax.numpy as jnp
    from jax.sharding import Mesh, PartitionSpec, NamedSharding
    from jax.experimental.shard_map import shard_map
    from concourse.bass2jax import (
        _bass_exec_p,
        partition_id_tensor,
        install_neuronx_cc_hook,
    )

    nc = _build_bass()
    install_neuronx_cc_hook()

    partition_name = nc.partition_id_tensor.name if nc.partition_id_tensor else None
    in_names, out_names, out_avals = [], [], []
    for alloc in nc.m.functions[0].allocations:
        if not isinstance(alloc, mybir.MemoryLocationSet):
            continue
        name = alloc.memorylocations[0].name
        if alloc.kind == "ExternalInput":
            if name != partition_name:
                in_names.append(name)
        elif alloc.kind == "ExternalOutput":
            shape = tuple(alloc.tensor_shape)
            dtype = mybir.dt.np(alloc.dtype)
            out_names.append(name)
            out_avals.append(jax.core.ShapedArray(shape, dtype))
    n_params = len(in_names)
    n_outs = len(out_avals)
    in_names_full = in_names + out_names + (
        [partition_name] if partition_name else []
    )
    donate = tuple(range(n_params, n_params + n_outs))

    def _body(*args):
        operands = list(args)
        if partition_name is not None:
            operands.append(partition_id_tensor())
        outs = _bass_exec_p.bind(
            *operands,
            out_avals=tuple(out_avals),
            in_names=tuple(in_names_full),
            out_names=tuple(out_names),
            lowering_input_output_aliases=(),
            sim_require_finite=True,
            sim_require_nnan=True,
            nc=nc,
        )
        return tuple(outs)

    devices = jax.devices()[:N_CORES]
    assert len(devices) == N_CORES, (
        f"need {N_CORES} devices, have {len(jax.devices())}"
    )
    mesh = Mesh(np.asarray(devices), ("core",))
    sh = NamedSharding(mesh, PartitionSpec("core"))
    in_specs = (PartitionSpec("core"),) * (n_params + n_outs)
    out_specs = (PartitionSpec("core"),) * n_outs
    sharded = jax.jit(
        shard_map(
            _body, mesh=mesh, in_specs=in_specs, out_specs=out_specs, check_rep=False
        ),
        donate_argnums=donate,
        keep_unused=True,
    )
    # donated output buffers, created on device (nothing on the wire);
    # the kernel writes every output element so zeros vs junk is moot, but
    # zeros keep parity with the stock run_bass_kernel_spmd semantics.
    # One dispatch makes the buffers for all N_CHUNKS chunk calls.
    zeros_fn = jax.jit(
        lambda: tuple(
            jnp.zeros((N_CORES * a.shape[0],) + a.shape[1:], a.dtype)
            for _ in range(N_CHUNKS)
            for a in out_avals
        ),
        out_shardings=(sh,) * (n_outs * N_CHUNKS),
    )

    from concurrent.futures import ThreadPoolExecutor

    _STATE.update(
        nc=nc,
        jax=jax,
        sharded=sharded,
        zeros_fn=zeros_fn,
        n_outs=n_outs,
        sharding=sh,
        in_names=in_names,
        out_names=out_names,
        weights=None,
        scale=None,
        consts_dev=None,
        pool=ThreadPoolExecutor(_NWORK),
        xq_buf=np.empty((BC, XC), dtype=np.int8),
        tmp_buf=np.empty((BC, 128), dtype=np.float32),
        xg_buf=np.empty((BC, 4), dtype=np.float16),
        inv_vec=np.where(_VEL_COLS, 1.0 / S_VEL, 1.0 / S_DEFAULT).astype(
            np.float32
        ),
    )
    return _STATE


def _ensure_consts(st, inputs, s, sv):
    """Device-cache the const blob; re-ship only when weights/scales change."""
    w = {k: np.asarray(inputs[k]) for k in WEIGHT_NAMES}
    cached = st["weights"]
    if (
        cached is not None
        and st["scale"] == (s, sv)
        and all(np.array_equal(cached[k], w[k]) for k in WEIGHT_NAMES)
    ):
        return
    blob = _build_const_blob(w, s, sv)
    glob = np.ascontiguousarray(
        np.broadcast_to(blob, (N_CORES,) + blob.shape).reshape(
            N_CORES * 128, _CONST_COLS
        )
    )
    ch = np.zeros((2, 66), dtype=np.float16)
    ch[:, 0:64] = w["psW1"][16:18]
    ch[:, 64:66] = np.eye(2, dtype=np.float16)
    chg = np.ascontiguousarray(
        np.broadcast_to(ch, (N_CORES,) + ch.shape).reshape(N_CORES * 2, 66)
    )
    consts_dev = st["jax"].device_put(glob, st["sharding"])
    csth_dev = st["jax"].device_put(chg, st["sharding"])
    consts_dev.block_until_ready()
    csth_dev.block_until_ready()
    st["weights"] = w
    st["scale"] = (s, sv)
    st["consts_dev"] = consts_dev
    st["csth_dev"] = csth_dev


def _dispatch_chunks(st, x, noise, inputs, s, sv, inv, strict):
    """Pack + dispatch every chunk with scales (s, sv) / inverse-scale
    ``inv``.  Returns the list of per-chunk output tuples, or None if
    strict and a chunk exceeds the code range."""
    _ensure_consts(st, inputs, s, sv)
    zeros = st["zeros_fn"]()
    no = st["n_outs"]
    outs = []
    for c in range(N_CHUNKS):
        xq, xg, in_range = _pack_chunk(st, x, noise, c, inv)
        if strict and not in_range:
            return None
        args = {
            "xq": xq,
            "xg": xg,
            "consts": st["consts_dev"],
            "csth": st["csth_dev"],
        }
        outs.append(
            st["sharded"](
                *[args[n] for n in st["in_names"]],
                *zeros[c * no : (c + 1) * no],
            )
        )
    return outs


def _run_once(st, x, noise, inputs):
    outs = _dispatch_chunks(
        st, x, noise, inputs, S_DEFAULT, S_VEL, st["inv_vec"], strict=True
    )
    if outs is None:
        # batch exceeds the covered feature range: fall back to a single
        # widened uniform scale (consts re-fold + re-ship), redo every chunk.
        amax = max(_chunk_range(st, x, c) for c in range(N_CHUNKS))
        s = max(amax / 127.0, S_DEFAULT)
        outs = _dispatch_chunks(
            st, x, noise, inputs, s, s, 1.0 / s, strict=False
        )
    parts = [np.asarray(o[0]).astype(np.float32) for o in outs]
    return parts[0] if len(parts) == 1 else np.concatenate(parts, axis=0)


def kernel(**inputs):
    st = _ensure_state()
    x = np.asarray(inputs["x"])
    noise = np.asarray(inputs["noise"])
    try:
        return _run_once(st, x, noise, inputs)
    except Exception:
        # one retry: the axon-tunneled device occasionally reports a
        # transient unrecoverable-exec error that clears on the next run
        return _run_once(st, x, noise, inputs)
